# revision 2
# baseline (speedup 1.0000x reference)
"""Swin-style attention (B=64,N=512,C=768,H=12) on 8 TRN2 NeuronCores.

Strategy: pure data-parallel over batch (8 batches/core), no collectives.
Per core, one fused pipeline per batch:
  phase1: qkT = Wqk @ x^T (f32r matmuls), v = x @ Wv^T (natural layout,
          padded with a ones-column per head for fused softmax sums)
  attn:   per head h: sT[j,i] = kT^T@qT (K=64, f32r) -> +biasT (DVE)
          -> exp (ACT, bf16 out) -> oT~[d,i] & sums via [v|1] matmul (bf16)
          -> normalize rows by 1/sums (partition_broadcast + DVE mul)
  proj:   out = oT^T @ Wp^T + pb (bf16 matmul, f32r-grade accuracy not
          needed after softmax averaging)
Scale 1/8 is folded into the q-half of Wqk on the host; softmax runs
without max-subtraction (scores are O(1) by construction).
"""
import sys

sys.path.insert(0, "/opt/trn_rl_repo")
from contextlib import ExitStack

import ml_dtypes
import numpy as np

import concourse.bass as bass
import concourse.mybir as mybir
import concourse.tile as tile
from concourse import bacc
from concourse.bass_utils import run_bass_kernel_spmd

F32 = mybir.dt.float32
F32R = mybir.dt.float32r
BF16 = mybir.dt.bfloat16

B, N, C, H, HD = 64, 512, 768, 12, 64
NCORES = 8
BL = B // NCORES          # batches per core
T = BL * N                # tokens per core
KC = C // 128             # 6 contraction chunks
NJT = N // 128            # 4 key-side tiles
NIT = N // 128            # 4 query/token tiles
VP = H * (HD + 1)         # 780: v padded with ones column per head
Exp = mybir.ActivationFunctionType.Exp


def _build():
    nc = bacc.Bacc(target_bir_lowering=False)
    xT_d = nc.dram_tensor("xT", [C, T], F32R, kind="ExternalInput")
    wqk_d = nc.dram_tensor("wqk", [C, 2 * C], F32R, kind="ExternalInput")
    wv_d = nc.dram_tensor("wv", [C, C], F32R, kind="ExternalInput")
    wp_d = nc.dram_tensor("wp", [C, C], BF16, kind="ExternalInput")
    biasT_d = nc.dram_tensor("biasT", [H, NJT, 128, N], BF16, kind="ExternalInput")
    pb_d = nc.dram_tensor("pb", [1, C], F32, kind="ExternalInput")
    out_d = nc.dram_tensor("out", [T, C], F32, kind="ExternalOutput")

    with ExitStack() as ctx:
        tc = ctx.enter_context(tile.TileContext(nc))
        const = ctx.enter_context(tc.tile_pool(name="const", bufs=1))
        perb = ctx.enter_context(tc.tile_pool(name="perb", bufs=1))
        xt_pool = ctx.enter_context(tc.tile_pool(name="xt", bufs=2))
        pool_s = ctx.enter_context(tc.tile_pool(name="ssb", bufs=3))
        pool_p = ctx.enter_context(tc.tile_pool(name="pt", bufs=5))
        pool_r = ctx.enter_context(tc.tile_pool(name="rc", bufs=2))
        pool_o = ctx.enter_context(tc.tile_pool(name="osb", bufs=2))
        mm_ps = ctx.enter_context(tc.tile_pool(name="mmps", bufs=2, space="PSUM"))
        s_ps = ctx.enter_context(tc.tile_pool(name="sps", bufs=4, space="PSUM"))
        o_ps = ctx.enter_context(tc.tile_pool(name="ops", bufs=2, space="PSUM"))

        # ---- constants ----
        wqk = const.tile([128, KC, 2 * C], F32R)
        wv = const.tile([128, KC, C], F32R)
        wp = const.tile([128, KC, C], BF16)
        biasT = const.tile([128, H, NJT, N], BF16)
        pb_bc = const.tile([128, C], F32)
        for kc in range(KC):
            nc.sync.dma_start(out=wqk[:, kc, :], in_=wqk_d[kc * 128:(kc + 1) * 128, :])
            nc.sync.dma_start(out=wv[:, kc, :], in_=wv_d[kc * 128:(kc + 1) * 128, :])
            nc.sync.dma_start(out=wp[:, kc, :], in_=wp_d[kc * 128:(kc + 1) * 128, :])
        for h in range(H):
            nc.sync.dma_start(
                out=biasT[:, h, :, :],
                in_=biasT_d[h, :, :, :].rearrange("a p b -> p a b"),
            )
        nc.sync.dma_start(out=pb_bc, in_=pb_d[0:1, :].to_broadcast((128, C)))

        for b in range(BL):
            # ---- load x^T for this batch ----
            xTb = xt_pool.tile([128, KC, N], F32R)
            for kc in range(KC):
                nc.sync.dma_start(
                    out=xTb[:, kc, :],
                    in_=xT_d[kc * 128:(kc + 1) * 128, b * N:(b + 1) * N],
                )

            # ---- phase 1a: qkT[r, i] for r in 2C (q rows then k rows) ----
            qkT = perb.tile([128, 2 * H // 2, N], F32R)
            for rt in range(2 * H // 2):  # 12 r-tiles of 128
                ps = mm_ps.tile([128, N], F32, tag="mm")
                for kc in range(KC):
                    nc.tensor.matmul(
                        ps,
                        wqk[:, kc, rt * 128:(rt + 1) * 128],
                        xTb[:, kc, :],
                        start=(kc == 0),
                        stop=(kc == KC - 1),
                    )
                nc.scalar.copy(out=qkT[:, rt, :], in_=ps)

            # ---- phase 1b: v natural, padded with ones column per head ----
            v_pad = perb.tile([128, NIT, VP], BF16)
            ones_view = v_pad.rearrange("p a (h e) -> p (a h) e", e=HD + 1)
            nc.vector.memset(ones_view[:, :, HD:HD + 1], 1.0)
            for it in range(NIT):
                for nh in range(2):
                    ps = mm_ps.tile([128, C // 2], F32, tag="mm")
                    for kc in range(KC):
                        nc.tensor.matmul(
                            ps,
                            xTb[:, kc, it * 128:(it + 1) * 128],
                            wv[:, kc, nh * 384:(nh + 1) * 384],
                            start=(kc == 0),
                            stop=(kc == KC - 1),
                        )
                    dest = v_pad[:, it, :].rearrange("p (h e) -> p h e", e=HD + 1)
                    nc.vector.tensor_copy(
                        out=dest[:, nh * 6:(nh + 1) * 6, 0:HD],
                        in_=ps.rearrange("p (h e) -> p h e", e=HD),
                    )

            # ---- attention per head ----
            oT = perb.tile([128, KC, N], BF16)
            for h in range(H):
                po = (h % 2) * 64
                rq, rk = h // 2, H // 2 + h // 2
                qT = qkT[po:po + 64, rq, :]
                pts = []
                for jt in range(NJT):
                    ps = s_ps.tile([128, N], F32, tag="sT")
                    nc.tensor.matmul(
                        ps,
                        qkT[po:po + 64, rk, jt * 128:(jt + 1) * 128],
                        qT,
                        start=True,
                        stop=True,
                    )
                    ssb = pool_s.tile([128, N], F32, tag="ssb")
                    nc.vector.tensor_add(ssb, ps, biasT[:, h, jt, :])
                    pt = pool_p.tile([128, N], BF16, tag="pT")
                    nc.scalar.activation(out=pt, in_=ssb, func=Exp)
                    pts.append(pt)
                pso = o_ps.tile([HD + 1, N], F32, tag="oT")
                for jt in range(NJT):
                    vp = v_pad[:, jt, :].rearrange("p (h e) -> p h e", e=HD + 1)
                    nc.tensor.matmul(
                        pso,
                        vp[:, h, :],
                        pts[jt],
                        start=(jt == 0),
                        stop=(jt == NJT - 1),
                    )
                rc1 = pool_r.tile([1, N], F32, tag="rc1")
                nc.vector.reciprocal(rc1, pso[HD:HD + 1, :])
                rc64 = pool_r.tile([64, N], F32, tag="rc64")
                nc.gpsimd.partition_broadcast(rc64, rc1)
                nc.vector.tensor_mul(oT[po:po + 64, rq, :], pso[0:HD, :], rc64)

            # ---- projection + bias + store ----
            for it in range(NIT):
                outsb = pool_o.tile([128, C], F32, tag="outsb")
                for ct in range(2):
                    ps = mm_ps.tile([128, C // 2], F32, tag="mm")
                    for kc in range(KC):
                        nc.tensor.matmul(
                            ps,
                            oT[:, kc, it * 128:(it + 1) * 128],
                            wp[:, kc, ct * 384:(ct + 1) * 384],
                            start=(kc == 0),
                            stop=(kc == KC - 1),
                        )
                    nc.vector.tensor_add(
                        outsb[:, ct * 384:(ct + 1) * 384],
                        ps,
                        pb_bc[:, ct * 384:(ct + 1) * 384],
                    )
                nc.sync.dma_start(
                    out=out_d[b * N + it * 128: b * N + (it + 1) * 128, :],
                    in_=outsb,
                )
    nc.finalize()
    return nc


def kernel(x, qkv_w, proj_w, proj_b, bias_table, _trace=False, _tmpdir=None):
    x = np.asarray(x, dtype=np.float32)
    qkv_w = np.asarray(qkv_w, dtype=np.float32)
    proj_w = np.asarray(proj_w, dtype=np.float32)
    proj_b = np.asarray(proj_b, dtype=np.float32)
    bias_table = np.asarray(bias_table, dtype=np.float32)

    # host-side layout prep (weights + bias table expansion)
    wq_scaled = qkv_w.copy()
    wq_scaled[:C] *= HD ** (-0.5)
    wqk = np.ascontiguousarray(wq_scaled[: 2 * C].T)              # [768, 1536]
    wv = np.ascontiguousarray(qkv_w[2 * C:].T)                    # [768, 768]
    wp = np.ascontiguousarray(proj_w.T).astype(ml_dtypes.bfloat16)
    ii = np.arange(N)
    idx = ii[None, :] - ii[:, None] + (N - 1)                     # [j, i]
    biasT = np.ascontiguousarray(
        bias_table[idx].transpose(2, 0, 1).reshape(H, NJT, 128, N)
    ).astype(ml_dtypes.bfloat16)
    pb = proj_b.reshape(1, C)

    nc = _build()
    in_maps = []
    for m in range(NCORES):
        xs = x[m * BL:(m + 1) * BL]                               # [8, 512, 768]
        xT = np.ascontiguousarray(xs.transpose(2, 0, 1).reshape(C, T))
        in_maps.append(
            {"xT": xT, "wqk": wqk, "wv": wv, "wp": wp, "biasT": biasT, "pb": pb}
        )
    res = run_bass_kernel_spmd(
        nc, in_maps, core_ids=list(range(NCORES)), trace=_trace, tmpdir=_tmpdir
    )
    out = np.concatenate(
        [res.results[m]["out"].reshape(BL, N, C) for m in range(NCORES)], axis=0
    )
    if _trace:
        return out, res
    return out


# revision 17
# speedup vs baseline: 1.2015x; 1.2015x over previous
"""Swin-style attention (B=64,N=512,C=768,H=12) on 8 TRN2 NeuronCores.

Strategy: pure data-parallel over batch (8 batches/core), no collectives.
Per core, one fused pipeline per batch:
  phase1: qkT = Wqk @ x^T (f32r matmuls), v = x @ Wv^T (natural layout,
          padded with a ones-column per head for fused softmax sums)
  attn:   per head h: sT[j,i] = kT^T@qT (K=64, f32r) -> +biasT (DVE)
          -> exp (ACT, bf16 out) -> oT~[d,i] & sums via [v|1] matmul (bf16)
          -> normalize rows by 1/sums (partition_broadcast + DVE mul)
  proj:   out = oT^T @ Wp^T + pb (bf16 matmul, f32r-grade accuracy not
          needed after softmax averaging)
Scale 1/8 is folded into the q-half of Wqk on the host; softmax runs
without max-subtraction (scores are O(1) by construction).
"""
import sys

sys.path.insert(0, "/opt/trn_rl_repo")
from contextlib import ExitStack

import ml_dtypes
import numpy as np

import concourse.bass as bass
import concourse.mybir as mybir
import concourse.tile as tile
from concourse import bacc
from concourse.bass_utils import run_bass_kernel_spmd
from concourse.masks import make_identity

F32 = mybir.dt.float32
F32R = mybir.dt.float32r
BF16 = mybir.dt.bfloat16

B, N, C, H, HD = 64, 512, 768, 12, 64
NCORES = 8
BL = B // NCORES          # batches per core
T = BL * N                # tokens per core
KC = C // 128             # 6 contraction chunks
NJT = N // 128            # 4 key-side tiles
NIT = N // 128            # 4 query/token tiles
VP = H * (HD + 1)         # 780: v padded with ones column per head
Exp = mybir.ActivationFunctionType.Exp


def _build():
    nc = bacc.Bacc(target_bir_lowering=False)
    xT_d = nc.dram_tensor("xT", [C, T], F32R, kind="ExternalInput")
    wqk_d = nc.dram_tensor("wqk", [C, 2 * C], F32R, kind="ExternalInput")
    wv_d = nc.dram_tensor("wv", [C, C], F32R, kind="ExternalInput")
    wp_d = nc.dram_tensor("wp", [C, C], BF16, kind="ExternalInput")
    biasT_d = nc.dram_tensor("biasT", [H, NJT, 128, N], BF16, kind="ExternalInput")
    pb_d = nc.dram_tensor("pb", [1, C], F32, kind="ExternalInput")
    out_d = nc.dram_tensor("out", [T, C], F32, kind="ExternalOutput")

    with ExitStack() as ctx:
        tc = ctx.enter_context(tile.TileContext(nc))
        const = ctx.enter_context(tc.tile_pool(name="const", bufs=1))
        perb = ctx.enter_context(tc.tile_pool(name="perb", bufs=1))
        xt_pool = ctx.enter_context(tc.tile_pool(name="xt", bufs=2))
        pool_p = ctx.enter_context(tc.tile_pool(name="pt", bufs=5))
        pool_r = ctx.enter_context(tc.tile_pool(name="rc", bufs=3))
        pool_o = ctx.enter_context(tc.tile_pool(name="osb", bufs=2))
        dram_p = ctx.enter_context(tc.tile_pool(name="dramp", bufs=2, space="DRAM"))
        mm_ps = ctx.enter_context(tc.tile_pool(name="mmps", bufs=2, space="PSUM"))
        s_ps = ctx.enter_context(tc.tile_pool(name="sps", bufs=4, space="PSUM"))
        o_ps = ctx.enter_context(tc.tile_pool(name="ops", bufs=2, space="PSUM"))

        # ---- constants ----
        wqk = const.tile([128, KC, 2 * C], F32R)
        wv = const.tile([128, KC, C], F32R)
        wp = const.tile([128, KC, C], BF16)
        biasT = const.tile([128, H, NJT, N], BF16)
        pb_bc = const.tile([128, C], F32)
        ident = const.tile([128, 128], BF16)
        make_identity(nc, ident)
        for kc in range(KC):
            nc.sync.dma_start(out=wqk[:, kc, :], in_=wqk_d[kc * 128:(kc + 1) * 128, :])
            nc.sync.dma_start(out=wv[:, kc, :], in_=wv_d[kc * 128:(kc + 1) * 128, :])
            nc.sync.dma_start(out=wp[:, kc, :], in_=wp_d[kc * 128:(kc + 1) * 128, :])
        for h in range(H):
            nc.sync.dma_start(
                out=biasT[:, h, :, :],
                in_=biasT_d[h, :, :, :].rearrange("a p b -> p a b"),
            )
        nc.sync.dma_start(out=pb_bc, in_=pb_d[0:1, :].to_broadcast((128, C)))

        for b in range(BL):
            # ---- load x^T for this batch ----
            xTb = xt_pool.tile([128, KC, N], F32R)
            for kc in range(KC):
                nc.sync.dma_start(
                    out=xTb[:, kc, :],
                    in_=xT_d[kc * 128:(kc + 1) * 128, b * N:(b + 1) * N],
                )

            # ---- phase 1a: qkT[r, i] for r in 2C (q rows then k rows) ----
            qkT = perb.tile([128, 2 * H // 2, N], F32R)
            for rt in range(2 * H // 2):  # 12 r-tiles of 128
                ps = mm_ps.tile([128, N], F32, tag="mm")
                for kc in range(KC):
                    nc.tensor.matmul(
                        ps,
                        wqk[:, kc, rt * 128:(rt + 1) * 128],
                        xTb[:, kc, :],
                        start=(kc == 0),
                        stop=(kc == KC - 1),
                    )
                nc.vector.tensor_copy(out=qkT[:, rt, :], in_=ps)

            # ---- phase 1b: v natural, padded with ones column per head ----
            v_pad = perb.tile([128, NIT, VP], BF16)
            ones_view = v_pad.rearrange("p a (h e) -> p (a h) e", e=HD + 1)
            nc.vector.memset(ones_view[:, :, HD:HD + 1], 1.0)
            for it in range(NIT):
                for nh in range(2):
                    ps = mm_ps.tile([128, C // 2], F32, tag="mm")
                    for kc in range(KC):
                        nc.tensor.matmul(
                            ps,
                            xTb[:, kc, it * 128:(it + 1) * 128],
                            wv[:, kc, nh * 384:(nh + 1) * 384],
                            start=(kc == 0),
                            stop=(kc == KC - 1),
                        )
                    dest = v_pad[:, it, :].rearrange("p (h e) -> p h e", e=HD + 1)
                    nc.vector.tensor_copy(
                        out=dest[:, nh * 6:(nh + 1) * 6, 0:HD],
                        in_=ps.rearrange("p (h e) -> p h e", e=HD),
                    )

            # ---- attention per head ----
            # scores: sT = kT^T@qT accumulated with I^T@biasT (bias add on PE);
            # exp straight from PSUM; softmax sums ride as row 64 of the
            # o-matmul via the ones column of v_pad.  Normalization is
            # deferred: unnormalized oT rows are stored bf16, reciprocals of
            # all 12 heads' sums are computed in one batched approx-recip per
            # batch, then applied in-place on gpsimd.
            oT = perb.tile([128, KC, N], BF16)
            for h in range(H):
                po = (h % 2) * 64
                rq, rk = h // 2, H // 2 + h // 2
                qT = qkT[po:po + 64, rq, :]
                pss = []
                for jt in range(NJT):
                    ps = s_ps.tile([128, N], F32, tag="sT")
                    nc.tensor.matmul(
                        ps,
                        qkT[po:po + 64, rk, jt * 128:(jt + 1) * 128],
                        qT,
                        start=True,
                        stop=False,
                    )
                    pss.append(ps)
                for jt in range(NJT):
                    nc.tensor.matmul(
                        pss[jt], ident, biasT[:, h, jt, :], start=False, stop=True
                    )
                pts = []
                for jt in range(NJT):
                    pt = pool_p.tile([128, N], BF16, tag="pT")
                    nc.scalar.activation(out=pt, in_=pss[jt], func=Exp)
                    pts.append(pt)
                pso = o_ps.tile([HD + 1, N], F32, tag="oT")
                for jt in range(NJT):
                    vp = v_pad[:, jt, :].rearrange("p (h e) -> p h e", e=HD + 1)
                    nc.tensor.matmul(
                        pso,
                        vp[:, h, :],
                        pts[jt],
                        start=(jt == 0),
                        stop=(jt == NJT - 1),
                    )
                nc.vector.tensor_copy(out=oT[po:po + 64, rq, :], in_=pso[0:HD, :])
                smc = pool_r.tile([65, N], F32, tag="smc")
                nc.scalar.copy(out=smc[HD:HD + 1, :], in_=pso[HD:HD + 1, :])
                rcd = dram_p.tile([1, N], F32, tag="rcd")
                nc.sync.dma_start(out=rcd, in_=smc[HD:HD + 1, :])
                rcf = pool_r.tile([128, N], F32, tag="rcf")
                nc.sync.dma_start(out=rcf, in_=rcd[0:1, :].to_broadcast((128, N)))
                nc.vector.reciprocal_approx_fast(out=rcf, in_=rcf)
                nc.vector.tensor_mul(
                    oT[po:po + 64, rq, :], oT[po:po + 64, rq, :],
                    rcf[po:po + 64, :],
                )

            # ---- projection + bias + store ----
            for it in range(NIT):
                outsb = pool_o.tile([128, C], F32, tag="outsb")
                for ct in range(2):
                    ps = mm_ps.tile([128, C // 2], F32, tag="mm")
                    for kc in range(KC):
                        nc.tensor.matmul(
                            ps,
                            oT[:, kc, it * 128:(it + 1) * 128],
                            wp[:, kc, ct * 384:(ct + 1) * 384],
                            start=(kc == 0),
                            stop=(kc == KC - 1),
                        )
                    nc.vector.tensor_add(
                        outsb[:, ct * 384:(ct + 1) * 384],
                        ps,
                        pb_bc[:, ct * 384:(ct + 1) * 384],
                    )
                nc.sync.dma_start(
                    out=out_d[b * N + it * 128: b * N + (it + 1) * 128, :],
                    in_=outsb,
                )
    nc.finalize()
    return nc


def kernel(x, qkv_w, proj_w, proj_b, bias_table, _trace=False, _tmpdir=None):
    x = np.asarray(x, dtype=np.float32)
    qkv_w = np.asarray(qkv_w, dtype=np.float32)
    proj_w = np.asarray(proj_w, dtype=np.float32)
    proj_b = np.asarray(proj_b, dtype=np.float32)
    bias_table = np.asarray(bias_table, dtype=np.float32)

    # host-side layout prep (weights + bias table expansion)
    wq_scaled = qkv_w.copy()
    wq_scaled[:C] *= HD ** (-0.5)
    wqk = np.ascontiguousarray(wq_scaled[: 2 * C].T)              # [768, 1536]
    wv = np.ascontiguousarray(qkv_w[2 * C:].T)                    # [768, 768]
    wp = np.ascontiguousarray(proj_w.T).astype(ml_dtypes.bfloat16)
    ii = np.arange(N)
    idx = ii[None, :] - ii[:, None] + (N - 1)                     # [j, i]
    biasT = np.ascontiguousarray(
        bias_table[idx].transpose(2, 0, 1).reshape(H, NJT, 128, N)
    ).astype(ml_dtypes.bfloat16)
    pb = proj_b.reshape(1, C)

    nc = _build()
    in_maps = []
    for m in range(NCORES):
        xs = x[m * BL:(m + 1) * BL]                               # [8, 512, 768]
        xT = np.ascontiguousarray(xs.transpose(2, 0, 1).reshape(C, T))
        in_maps.append(
            {"xT": xT, "wqk": wqk, "wv": wv, "wp": wp, "biasT": biasT, "pb": pb}
        )
    res = run_bass_kernel_spmd(
        nc, in_maps, core_ids=list(range(NCORES)), trace=_trace, tmpdir=_tmpdir
    )
    out = np.concatenate(
        [res.results[m]["out"].reshape(BL, N, C) for m in range(NCORES)], axis=0
    )
    if _trace:
        return out, res
    return out


# revision 18
# speedup vs baseline: 1.4888x; 1.2391x over previous
"""Swin-style attention (B=64,N=512,C=768,H=12) on 8 TRN2 NeuronCores.

Strategy: pure data-parallel over batch (8 batches/core), no collectives.
Per core, one fused pipeline per batch:
  phase1: qkT = Wqk @ x^T (f32r matmuls), v = x @ Wv^T (natural layout,
          padded with a ones-column per head for fused softmax sums)
  attn:   per head h: sT[j,i] = kT^T@qT (K=64, f32r) -> +biasT (DVE)
          -> exp (ACT, bf16 out) -> oT~[d,i] & sums via [v|1] matmul (bf16)
          -> normalize rows by 1/sums (partition_broadcast + DVE mul)
  proj:   out = oT^T @ Wp^T + pb (bf16 matmul, f32r-grade accuracy not
          needed after softmax averaging)
Scale 1/8 is folded into the q-half of Wqk on the host; softmax runs
without max-subtraction (scores are O(1) by construction).
"""
import sys

sys.path.insert(0, "/opt/trn_rl_repo")
from contextlib import ExitStack

import ml_dtypes
import numpy as np

import concourse.bass as bass
import concourse.mybir as mybir
import concourse.tile as tile
from concourse import bacc
from concourse.bass_utils import run_bass_kernel_spmd
from concourse.masks import make_identity

F32 = mybir.dt.float32
F32R = mybir.dt.float32r
BF16 = mybir.dt.bfloat16

B, N, C, H, HD = 64, 512, 768, 12, 64
NCORES = 8
BL = B // NCORES          # batches per core
T = BL * N                # tokens per core
KC = C // 128             # 6 contraction chunks
NJT = N // 128            # 4 key-side tiles
NIT = N // 128            # 4 query/token tiles
VP = H * (HD + 1)         # 780: v padded with ones column per head
Exp = mybir.ActivationFunctionType.Exp


def _build():
    nc = bacc.Bacc(target_bir_lowering=False)
    xT_d = nc.dram_tensor("xT", [C, T], BF16, kind="ExternalInput")
    wqk_d = nc.dram_tensor("wqk", [C, 2 * C], BF16, kind="ExternalInput")
    wv_d = nc.dram_tensor("wv", [C, C], BF16, kind="ExternalInput")
    wp_d = nc.dram_tensor("wp", [C, C], BF16, kind="ExternalInput")
    biasT_d = nc.dram_tensor("biasT", [H, NJT, 128, N], BF16, kind="ExternalInput")
    pb_d = nc.dram_tensor("pb", [1, C], F32, kind="ExternalInput")
    out_d = nc.dram_tensor("out", [T, C], F32, kind="ExternalOutput")

    with ExitStack() as ctx:
        tc = ctx.enter_context(tile.TileContext(nc))
        const = ctx.enter_context(tc.tile_pool(name="const", bufs=1))
        perb = ctx.enter_context(tc.tile_pool(name="perb", bufs=1))
        xt_pool = ctx.enter_context(tc.tile_pool(name="xt", bufs=2))
        pool_p = ctx.enter_context(tc.tile_pool(name="pt", bufs=5))
        pool_r = ctx.enter_context(tc.tile_pool(name="rc", bufs=3))
        pool_o = ctx.enter_context(tc.tile_pool(name="osb", bufs=2))
        dram_p = ctx.enter_context(tc.tile_pool(name="dramp", bufs=2, space="DRAM"))
        mm_ps = ctx.enter_context(tc.tile_pool(name="mmps", bufs=2, space="PSUM"))
        s_ps = ctx.enter_context(tc.tile_pool(name="sps", bufs=4, space="PSUM"))
        o_ps = ctx.enter_context(tc.tile_pool(name="ops", bufs=2, space="PSUM"))

        # ---- constants ----
        wqk = const.tile([128, KC, 2 * C], BF16)
        wv = const.tile([128, KC, C], BF16)
        wp = const.tile([128, KC, C], BF16)
        biasT = const.tile([128, H, NJT, N], BF16)
        pb_bc = const.tile([128, C], F32)
        ident = const.tile([128, 128], BF16)
        make_identity(nc, ident)
        for kc in range(KC):
            nc.sync.dma_start(out=wqk[:, kc, :], in_=wqk_d[kc * 128:(kc + 1) * 128, :])
            nc.sync.dma_start(out=wv[:, kc, :], in_=wv_d[kc * 128:(kc + 1) * 128, :])
            nc.sync.dma_start(out=wp[:, kc, :], in_=wp_d[kc * 128:(kc + 1) * 128, :])
        for h in range(H):
            nc.sync.dma_start(
                out=biasT[:, h, :, :],
                in_=biasT_d[h, :, :, :].rearrange("a p b -> p a b"),
            )
        nc.sync.dma_start(out=pb_bc, in_=pb_d[0:1, :].to_broadcast((128, C)))

        for b in range(BL):
            # ---- load x^T for this batch ----
            xTb = xt_pool.tile([128, KC, N], BF16)
            for kc in range(KC):
                nc.sync.dma_start(
                    out=xTb[:, kc, :],
                    in_=xT_d[kc * 128:(kc + 1) * 128, b * N:(b + 1) * N],
                )

            # ---- phase 1a: qkT[r, i] for r in 2C (q rows then k rows) ----
            qkT = perb.tile([128, 2 * H // 2, N], BF16)
            for rt in range(2 * H // 2):  # 12 r-tiles of 128
                ps = mm_ps.tile([128, N], F32, tag="mm")
                for kc in range(KC):
                    nc.tensor.matmul(
                        ps,
                        wqk[:, kc, rt * 128:(rt + 1) * 128],
                        xTb[:, kc, :],
                        start=(kc == 0),
                        stop=(kc == KC - 1),
                    )
                nc.vector.tensor_copy(out=qkT[:, rt, :], in_=ps)

            # ---- phase 1b: v natural, padded with ones column per head ----
            v_pad = perb.tile([128, NIT, VP], BF16)
            ones_view = v_pad.rearrange("p a (h e) -> p (a h) e", e=HD + 1)
            nc.vector.memset(ones_view[:, :, HD:HD + 1], 1.0)
            for it in range(NIT):
                for nh in range(2):
                    ps = mm_ps.tile([128, C // 2], F32, tag="mm")
                    for kc in range(KC):
                        nc.tensor.matmul(
                            ps,
                            xTb[:, kc, it * 128:(it + 1) * 128],
                            wv[:, kc, nh * 384:(nh + 1) * 384],
                            start=(kc == 0),
                            stop=(kc == KC - 1),
                        )
                    dest = v_pad[:, it, :].rearrange("p (h e) -> p h e", e=HD + 1)
                    nc.vector.tensor_copy(
                        out=dest[:, nh * 6:(nh + 1) * 6, 0:HD],
                        in_=ps.rearrange("p (h e) -> p h e", e=HD),
                    )

            # ---- attention per head ----
            # scores: sT = kT^T@qT accumulated with I^T@biasT (bias add on PE);
            # exp straight from PSUM; softmax sums ride as row 64 of the
            # o-matmul via the ones column of v_pad.  Normalization is
            # deferred: unnormalized oT rows are stored bf16, reciprocals of
            # all 12 heads' sums are computed in one batched approx-recip per
            # batch, then applied in-place on gpsimd.
            oT = perb.tile([128, KC, N], BF16)
            for h in range(H):
                po = (h % 2) * 64
                rq, rk = h // 2, H // 2 + h // 2
                qT = qkT[po:po + 64, rq, :]
                pss = []
                for jt in range(NJT):
                    ps = s_ps.tile([128, N], F32, tag="sT")
                    nc.tensor.matmul(
                        ps,
                        qkT[po:po + 64, rk, jt * 128:(jt + 1) * 128],
                        qT,
                        start=True,
                        stop=False,
                    )
                    pss.append(ps)
                for jt in range(NJT):
                    nc.tensor.matmul(
                        pss[jt], ident, biasT[:, h, jt, :], start=False, stop=True
                    )
                pts = []
                for jt in range(NJT):
                    pt = pool_p.tile([128, N], BF16, tag="pT")
                    nc.scalar.activation(out=pt, in_=pss[jt], func=Exp)
                    pts.append(pt)
                pso = o_ps.tile([HD + 1, N], F32, tag="oT")
                for jt in range(NJT):
                    vp = v_pad[:, jt, :].rearrange("p (h e) -> p h e", e=HD + 1)
                    nc.tensor.matmul(
                        pso,
                        vp[:, h, :],
                        pts[jt],
                        start=(jt == 0),
                        stop=(jt == NJT - 1),
                    )
                nc.vector.tensor_copy(out=oT[po:po + 64, rq, :], in_=pso[0:HD, :])
                smc = pool_r.tile([65, N], F32, tag="smc")
                nc.scalar.copy(out=smc[HD:HD + 1, :], in_=pso[HD:HD + 1, :])
                rcd = dram_p.tile([1, N], F32, tag="rcd")
                nc.sync.dma_start(out=rcd, in_=smc[HD:HD + 1, :])
                rcf = pool_r.tile([128, N], F32, tag="rcf")
                nc.sync.dma_start(out=rcf, in_=rcd[0:1, :].to_broadcast((128, N)))
                nc.vector.reciprocal_approx_fast(out=rcf, in_=rcf)
                nc.vector.tensor_mul(
                    oT[po:po + 64, rq, :], oT[po:po + 64, rq, :],
                    rcf[po:po + 64, :],
                )

            # ---- projection + bias + store ----
            for it in range(NIT):
                outsb = pool_o.tile([128, C], F32, tag="outsb")
                for ct in range(2):
                    ps = mm_ps.tile([128, C // 2], F32, tag="mm")
                    for kc in range(KC):
                        nc.tensor.matmul(
                            ps,
                            oT[:, kc, it * 128:(it + 1) * 128],
                            wp[:, kc, ct * 384:(ct + 1) * 384],
                            start=(kc == 0),
                            stop=(kc == KC - 1),
                        )
                    nc.vector.tensor_add(
                        outsb[:, ct * 384:(ct + 1) * 384],
                        ps,
                        pb_bc[:, ct * 384:(ct + 1) * 384],
                    )
                nc.sync.dma_start(
                    out=out_d[b * N + it * 128: b * N + (it + 1) * 128, :],
                    in_=outsb,
                )
    nc.finalize()
    return nc


def kernel(x, qkv_w, proj_w, proj_b, bias_table, _trace=False, _tmpdir=None):
    x = np.asarray(x, dtype=np.float32)
    qkv_w = np.asarray(qkv_w, dtype=np.float32)
    proj_w = np.asarray(proj_w, dtype=np.float32)
    proj_b = np.asarray(proj_b, dtype=np.float32)
    bias_table = np.asarray(bias_table, dtype=np.float32)

    # host-side layout prep (weights + bias table expansion)
    wq_scaled = qkv_w.copy()
    wq_scaled[:C] *= HD ** (-0.5)
    wqk = np.ascontiguousarray(wq_scaled[: 2 * C].T).astype(ml_dtypes.bfloat16)
    wv = np.ascontiguousarray(qkv_w[2 * C:].T).astype(ml_dtypes.bfloat16)
    wp = np.ascontiguousarray(proj_w.T).astype(ml_dtypes.bfloat16)
    ii = np.arange(N)
    idx = ii[None, :] - ii[:, None] + (N - 1)                     # [j, i]
    biasT = np.ascontiguousarray(
        bias_table[idx].transpose(2, 0, 1).reshape(H, NJT, 128, N)
    ).astype(ml_dtypes.bfloat16)
    pb = proj_b.reshape(1, C)

    nc = _build()
    in_maps = []
    for m in range(NCORES):
        xs = x[m * BL:(m + 1) * BL]                               # [8, 512, 768]
        xT = np.ascontiguousarray(xs.transpose(2, 0, 1).reshape(C, T)).astype(ml_dtypes.bfloat16)
        in_maps.append(
            {"xT": xT, "wqk": wqk, "wv": wv, "wp": wp, "biasT": biasT, "pb": pb}
        )
    res = run_bass_kernel_spmd(
        nc, in_maps, core_ids=list(range(NCORES)), trace=_trace, tmpdir=_tmpdir
    )
    out = np.concatenate(
        [res.results[m]["out"].reshape(BL, N, C) for m in range(NCORES)], axis=0
    )
    if _trace:
        return out, res
    return out


# revision 19
# speedup vs baseline: 1.5492x; 1.0406x over previous
"""Swin-style attention (B=64,N=512,C=768,H=12) on 8 TRN2 NeuronCores.

Strategy: pure data-parallel over batch (8 batches/core), no collectives.
Per core, one fused pipeline per batch:
  phase1: qkT = Wqk @ x^T (f32r matmuls), v = x @ Wv^T (natural layout,
          padded with a ones-column per head for fused softmax sums)
  attn:   per head h: sT[j,i] = kT^T@qT (K=64, f32r) -> +biasT (DVE)
          -> exp (ACT, bf16 out) -> oT~[d,i] & sums via [v|1] matmul (bf16)
          -> normalize rows by 1/sums (partition_broadcast + DVE mul)
  proj:   out = oT^T @ Wp^T + pb (bf16 matmul, f32r-grade accuracy not
          needed after softmax averaging)
Scale 1/8 is folded into the q-half of Wqk on the host; softmax runs
without max-subtraction (scores are O(1) by construction).
"""
import sys

sys.path.insert(0, "/opt/trn_rl_repo")
from contextlib import ExitStack

import ml_dtypes
import numpy as np

import concourse.bass as bass
import concourse.mybir as mybir
import concourse.tile as tile
from concourse import bacc
from concourse.bass_utils import run_bass_kernel_spmd
from concourse.masks import make_identity

F32 = mybir.dt.float32
F32R = mybir.dt.float32r
BF16 = mybir.dt.bfloat16

B, N, C, H, HD = 64, 512, 768, 12, 64
NCORES = 8
BL = B // NCORES          # batches per core
T = BL * N                # tokens per core
KC = C // 128             # 6 contraction chunks
NJT = N // 128            # 4 key-side tiles
NIT = N // 128            # 4 query/token tiles
VP = H * (HD + 1)         # 780: v padded with ones column per head
Exp = mybir.ActivationFunctionType.Exp


def _build():
    nc = bacc.Bacc(target_bir_lowering=False)
    xT_d = nc.dram_tensor("xT", [C, T], BF16, kind="ExternalInput")
    wqk_d = nc.dram_tensor("wqk", [C, 2 * C], BF16, kind="ExternalInput")
    wv_d = nc.dram_tensor("wv", [C, C], BF16, kind="ExternalInput")
    wp_d = nc.dram_tensor("wp", [C, C], BF16, kind="ExternalInput")
    biasT_d = nc.dram_tensor("biasT", [H, NJT, 128, N], BF16, kind="ExternalInput")
    pb_d = nc.dram_tensor("pb", [1, C], F32, kind="ExternalInput")
    out_d = nc.dram_tensor("out", [T, C], F32, kind="ExternalOutput")

    with ExitStack() as ctx:
        tc = ctx.enter_context(tile.TileContext(nc))
        const = ctx.enter_context(tc.tile_pool(name="const", bufs=1))
        perb = ctx.enter_context(tc.tile_pool(name="perb", bufs=2))
        perb1 = ctx.enter_context(tc.tile_pool(name="perb1", bufs=1))
        xt_pool = ctx.enter_context(tc.tile_pool(name="xt", bufs=2))
        pool_p = ctx.enter_context(tc.tile_pool(name="pt", bufs=5))
        pool_r = ctx.enter_context(tc.tile_pool(name="rc", bufs=3))
        pool_o = ctx.enter_context(tc.tile_pool(name="osb", bufs=2))
        dram_p = ctx.enter_context(tc.tile_pool(name="dramp", bufs=2, space="DRAM"))
        mm_ps = ctx.enter_context(tc.tile_pool(name="mmps", bufs=2, space="PSUM"))
        s_ps = ctx.enter_context(tc.tile_pool(name="sps", bufs=2, space="PSUM"))
        o_ps = ctx.enter_context(tc.tile_pool(name="ops", bufs=2, space="PSUM"))

        # ---- constants ----
        wqk = const.tile([128, KC, 2 * C], BF16)
        wv = const.tile([128, KC, C], BF16)
        wp = const.tile([128, KC, C], BF16)
        biasT = const.tile([128, H, NJT, N], BF16)
        pb_bc = const.tile([128, C], F32)
        ident = const.tile([128, 128], BF16)
        make_identity(nc, ident)
        for kc in range(KC):
            nc.sync.dma_start(out=wqk[:, kc, :], in_=wqk_d[kc * 128:(kc + 1) * 128, :])
            nc.sync.dma_start(out=wv[:, kc, :], in_=wv_d[kc * 128:(kc + 1) * 128, :])
            nc.sync.dma_start(out=wp[:, kc, :], in_=wp_d[kc * 128:(kc + 1) * 128, :])
        for h in range(H):
            nc.sync.dma_start(
                out=biasT[:, h, :, :],
                in_=biasT_d[h, :, :, :].rearrange("a p b -> p a b"),
            )
        nc.sync.dma_start(out=pb_bc, in_=pb_d[0:1, :].to_broadcast((128, C)))

        for b in range(BL):
            # ---- load x^T for this batch ----
            xTb = xt_pool.tile([128, KC, N], BF16)
            for kc in range(KC):
                nc.sync.dma_start(
                    out=xTb[:, kc, :],
                    in_=xT_d[kc * 128:(kc + 1) * 128, b * N:(b + 1) * N],
                )

            # ---- phase 1a: qkT[r, i] for r in 2C (q rows then k rows) ----
            qkT = perb.tile([128, 2 * H // 2, N], BF16)
            for rt in range(2 * H // 2):  # 12 r-tiles of 128
                ps = mm_ps.tile([128, N], F32, tag="mm")
                for kc in range(KC):
                    nc.tensor.matmul(
                        ps,
                        wqk[:, kc, rt * 128:(rt + 1) * 128],
                        xTb[:, kc, :],
                        start=(kc == 0),
                        stop=(kc == KC - 1),
                    )
                nc.vector.tensor_copy(out=qkT[:, rt, :], in_=ps)

            # ---- phase 1b: v natural, padded with ones column per head ----
            v_pad = perb.tile([128, NIT, VP], BF16)
            ones_view = v_pad.rearrange("p a (h e) -> p (a h) e", e=HD + 1)
            nc.vector.memset(ones_view[:, :, HD:HD + 1], 1.0)
            for it in range(NIT):
                for nh in range(2):
                    ps = mm_ps.tile([128, C // 2], F32, tag="mm")
                    for kc in range(KC):
                        nc.tensor.matmul(
                            ps,
                            xTb[:, kc, it * 128:(it + 1) * 128],
                            wv[:, kc, nh * 384:(nh + 1) * 384],
                            start=(kc == 0),
                            stop=(kc == KC - 1),
                        )
                    dest = v_pad[:, it, :].rearrange("p (h e) -> p h e", e=HD + 1)
                    nc.vector.tensor_copy(
                        out=dest[:, nh * 6:(nh + 1) * 6, 0:HD],
                        in_=ps.rearrange("p (h e) -> p h e", e=HD),
                    )

            # ---- attention per head ----
            # scores: sT = kT^T@qT accumulated with I^T@biasT (bias add on PE);
            # exp straight from PSUM; softmax sums ride as row 64 of the
            # o-matmul via the ones column of v_pad.  Normalization is
            # deferred: unnormalized oT rows are stored bf16, reciprocals of
            # all 12 heads' sums are computed in one batched approx-recip per
            # batch, then applied in-place on gpsimd.
            oT = perb1.tile([128, KC, N], BF16)
            for h in range(H):
                po = (h % 2) * 64
                rq, rk = h // 2, H // 2 + h // 2
                qT = qkT[po:po + 64, rq, :]
                pts = []
                for jp in range(NJT // 2):
                    ps2 = s_ps.tile([128, 2, N], F32, tag="sT")
                    for jl in range(2):
                        jt = jp * 2 + jl
                        nc.tensor.matmul(
                            ps2[:, jl, :],
                            qkT[po:po + 64, rk, jt * 128:(jt + 1) * 128],
                            qT,
                            start=True,
                            stop=False,
                        )
                    for jl in range(2):
                        jt = jp * 2 + jl
                        nc.tensor.matmul(
                            ps2[:, jl, :], ident, biasT[:, h, jt, :],
                            start=False, stop=True,
                        )
                    pt2 = pool_p.tile([128, 2, N], BF16, tag="pT")
                    nc.scalar.activation(out=pt2, in_=ps2, func=Exp)
                    pts.append(pt2)
                pso = o_ps.tile([HD + 1, N], F32, tag="oT")
                for jt in range(NJT):
                    vp = v_pad[:, jt, :].rearrange("p (h e) -> p h e", e=HD + 1)
                    nc.tensor.matmul(
                        pso,
                        vp[:, h, :],
                        pts[jt // 2][:, jt % 2, :],
                        start=(jt == 0),
                        stop=(jt == NJT - 1),
                    )
                nc.vector.tensor_copy(out=oT[po:po + 64, rq, :], in_=pso[0:HD, :])
                smc = pool_r.tile([65, N], F32, tag="smc")
                nc.vector.tensor_copy(out=smc[HD:HD + 1, :], in_=pso[HD:HD + 1, :])
                rcd = dram_p.tile([1, N], F32, tag="rcd")
                nc.sync.dma_start(out=rcd, in_=smc[HD:HD + 1, :])
                rcf = pool_r.tile([128, N], F32, tag="rcf")
                nc.sync.dma_start(out=rcf, in_=rcd[0:1, :].to_broadcast((128, N)))
                nc.vector.reciprocal_approx_fast(out=rcf, in_=rcf)
                nc.vector.tensor_mul(
                    oT[po:po + 64, rq, :], oT[po:po + 64, rq, :],
                    rcf[po:po + 64, :],
                )

            # ---- projection + bias + store ----
            for it in range(NIT):
                outsb = pool_o.tile([128, C], F32, tag="outsb")
                for ct in range(2):
                    ps = mm_ps.tile([128, C // 2], F32, tag="mm")
                    for kc in range(KC):
                        nc.tensor.matmul(
                            ps,
                            oT[:, kc, it * 128:(it + 1) * 128],
                            wp[:, kc, ct * 384:(ct + 1) * 384],
                            start=(kc == 0),
                            stop=(kc == KC - 1),
                        )
                    nc.vector.tensor_add(
                        outsb[:, ct * 384:(ct + 1) * 384],
                        ps,
                        pb_bc[:, ct * 384:(ct + 1) * 384],
                    )
                nc.sync.dma_start(
                    out=out_d[b * N + it * 128: b * N + (it + 1) * 128, :],
                    in_=outsb,
                )
    nc.finalize()
    return nc


def kernel(x, qkv_w, proj_w, proj_b, bias_table, _trace=False, _tmpdir=None):
    x = np.asarray(x, dtype=np.float32)
    qkv_w = np.asarray(qkv_w, dtype=np.float32)
    proj_w = np.asarray(proj_w, dtype=np.float32)
    proj_b = np.asarray(proj_b, dtype=np.float32)
    bias_table = np.asarray(bias_table, dtype=np.float32)

    # host-side layout prep (weights + bias table expansion)
    wq_scaled = qkv_w.copy()
    wq_scaled[:C] *= HD ** (-0.5)
    wqk = np.ascontiguousarray(wq_scaled[: 2 * C].T).astype(ml_dtypes.bfloat16)
    wv = np.ascontiguousarray(qkv_w[2 * C:].T).astype(ml_dtypes.bfloat16)
    wp = np.ascontiguousarray(proj_w.T).astype(ml_dtypes.bfloat16)
    ii = np.arange(N)
    idx = ii[None, :] - ii[:, None] + (N - 1)                     # [j, i]
    biasT = np.ascontiguousarray(
        bias_table[idx].transpose(2, 0, 1).reshape(H, NJT, 128, N)
    ).astype(ml_dtypes.bfloat16)
    pb = proj_b.reshape(1, C)

    nc = _build()
    in_maps = []
    for m in range(NCORES):
        xs = x[m * BL:(m + 1) * BL]                               # [8, 512, 768]
        xT = np.ascontiguousarray(xs.transpose(2, 0, 1).reshape(C, T)).astype(ml_dtypes.bfloat16)
        in_maps.append(
            {"xT": xT, "wqk": wqk, "wv": wv, "wp": wp, "biasT": biasT, "pb": pb}
        )
    res = run_bass_kernel_spmd(
        nc, in_maps, core_ids=list(range(NCORES)), trace=_trace, tmpdir=_tmpdir
    )
    out = np.concatenate(
        [res.results[m]["out"].reshape(BL, N, C) for m in range(NCORES)], axis=0
    )
    if _trace:
        return out, res
    return out


# revision 22
# speedup vs baseline: 1.6510x; 1.0658x over previous
"""Swin-style attention (B=64,N=512,C=768,H=12) on 8 TRN2 NeuronCores.

Strategy: pure data-parallel over batch (8 batches/core), no collectives.
Per core, one fused pipeline per batch:
  phase1: qkT = Wqk @ x^T (f32r matmuls), v = x @ Wv^T (natural layout,
          padded with a ones-column per head for fused softmax sums)
  attn:   per head h: sT[j,i] = kT^T@qT (K=64, f32r) -> +biasT (DVE)
          -> exp (ACT, bf16 out) -> oT~[d,i] & sums via [v|1] matmul (bf16)
          -> normalize rows by 1/sums (partition_broadcast + DVE mul)
  proj:   out = oT^T @ Wp^T + pb (bf16 matmul, f32r-grade accuracy not
          needed after softmax averaging)
Scale 1/8 is folded into the q-half of Wqk on the host; softmax runs
without max-subtraction (scores are O(1) by construction).
"""
import sys

sys.path.insert(0, "/opt/trn_rl_repo")
from contextlib import ExitStack

import ml_dtypes
import numpy as np

import concourse.bass as bass
import concourse.mybir as mybir
import concourse.tile as tile
from concourse import bacc
from concourse.bass_utils import run_bass_kernel_spmd
from concourse.masks import make_identity

F32 = mybir.dt.float32
F32R = mybir.dt.float32r
BF16 = mybir.dt.bfloat16

B, N, C, H, HD = 64, 512, 768, 12, 64
NCORES = 8
BL = B // NCORES          # batches per core
T = BL * N                # tokens per core
KC = C // 128             # 6 contraction chunks
NJT = N // 128            # 4 key-side tiles
NIT = N // 128            # 4 query/token tiles
VP = H * (HD + 1)         # 780: v padded with ones column per head
Exp = mybir.ActivationFunctionType.Exp


def _build():
    nc = bacc.Bacc(target_bir_lowering=False)
    xT_d = nc.dram_tensor("xT", [C, T], BF16, kind="ExternalInput")
    wqk_d = nc.dram_tensor("wqk", [C, 2 * C], BF16, kind="ExternalInput")
    wv_d = nc.dram_tensor("wv", [C, C], BF16, kind="ExternalInput")
    wp_d = nc.dram_tensor("wp", [C, C], BF16, kind="ExternalInput")
    biasT_d = nc.dram_tensor("biasT", [H, NJT, 128, N], BF16, kind="ExternalInput")
    pb_d = nc.dram_tensor("pb", [1, C], F32, kind="ExternalInput")
    out_d = nc.dram_tensor("out", [T, C], F32, kind="ExternalOutput")

    with ExitStack() as ctx:
        tc = ctx.enter_context(tile.TileContext(nc))
        const = ctx.enter_context(tc.tile_pool(name="const", bufs=1))
        perb = ctx.enter_context(tc.tile_pool(name="perb", bufs=2))
        perb1 = ctx.enter_context(tc.tile_pool(name="perb1", bufs=2))
        xt_pool = ctx.enter_context(tc.tile_pool(name="xt", bufs=2))
        pool_p = ctx.enter_context(tc.tile_pool(name="pt", bufs=5))
        pool_r = ctx.enter_context(tc.tile_pool(name="rc", bufs=3))
        pool_o = ctx.enter_context(tc.tile_pool(name="osb", bufs=2))
        dram_p = ctx.enter_context(tc.tile_pool(name="dramp", bufs=2, space="DRAM"))
        mm_ps = ctx.enter_context(tc.tile_pool(name="mmps", bufs=2, space="PSUM"))
        s_ps = ctx.enter_context(tc.tile_pool(name="sps", bufs=2, space="PSUM"))
        o_ps = ctx.enter_context(tc.tile_pool(name="ops", bufs=2, space="PSUM"))

        # ---- constants ----
        wqk = const.tile([128, KC, 2 * C], BF16)
        wv = const.tile([128, KC, C], BF16)
        wp = const.tile([128, KC, C], BF16)
        biasT = const.tile([128, H, NJT, N], BF16)
        pb_bc = const.tile([128, C], F32)
        ident = const.tile([128, 128], BF16)
        make_identity(nc, ident)
        for kc in range(KC):
            nc.sync.dma_start(out=wqk[:, kc, :], in_=wqk_d[kc * 128:(kc + 1) * 128, :])
            nc.sync.dma_start(out=wv[:, kc, :], in_=wv_d[kc * 128:(kc + 1) * 128, :])
            nc.sync.dma_start(out=wp[:, kc, :], in_=wp_d[kc * 128:(kc + 1) * 128, :])
        for h in range(H):
            nc.sync.dma_start(
                out=biasT[:, h, :, :],
                in_=biasT_d[h, :, :, :].rearrange("a p b -> p a b"),
            )
        nc.sync.dma_start(out=pb_bc, in_=pb_d[0:1, :].to_broadcast((128, C)))

        def phase1(b):
            # ---- load x^T for this batch ----
            xTb = xt_pool.tile([128, KC, N], BF16, tag="xTb")
            for kc in range(KC):
                nc.sync.dma_start(
                    out=xTb[:, kc, :],
                    in_=xT_d[kc * 128:(kc + 1) * 128, b * N:(b + 1) * N],
                )

            # ---- phase 1a: qkT[r, i] for r in 2C (q rows then k rows) ----
            qkT = perb.tile([128, 2 * H // 2, N], BF16, tag="qkT")
            for rt in range(2 * H // 2):  # 12 r-tiles of 128
                ps = mm_ps.tile([128, N], F32, tag="mm")
                for kc in range(KC):
                    nc.tensor.matmul(
                        ps,
                        wqk[:, kc, rt * 128:(rt + 1) * 128],
                        xTb[:, kc, :],
                        start=(kc == 0),
                        stop=(kc == KC - 1),
                    )
                nc.vector.tensor_copy(out=qkT[:, rt, :], in_=ps)

            # ---- phase 1b: v natural, padded with ones column per head ----
            v_pad = perb.tile([128, NIT, VP], BF16, tag="v_pad")
            ones_view = v_pad.rearrange("p a (h e) -> p (a h) e", e=HD + 1)
            nc.vector.memset(ones_view[:, :, HD:HD + 1], 1.0)
            for it in range(NIT):
                for nh in range(2):
                    ps = mm_ps.tile([128, C // 2], F32, tag="mm")
                    for kc in range(KC):
                        nc.tensor.matmul(
                            ps,
                            xTb[:, kc, it * 128:(it + 1) * 128],
                            wv[:, kc, nh * 384:(nh + 1) * 384],
                            start=(kc == 0),
                            stop=(kc == KC - 1),
                        )
                    dest = v_pad[:, it, :].rearrange("p (h e) -> p h e", e=HD + 1)
                    nc.vector.tensor_copy(
                        out=dest[:, nh * 6:(nh + 1) * 6, 0:HD],
                        in_=ps.rearrange("p (h e) -> p h e", e=HD),
                    )
            return qkT, v_pad

        def attention(b, qkT, v_pad):
            # scores: sT = kT^T@qT accumulated with I^T@biasT (bias add on PE);
            # exp straight from PSUM; softmax sums ride as row 64 of the
            # o-matmul via the ones column of v_pad.  Normalization is
            # deferred: unnormalized oT rows are stored bf16, per-head
            # reciprocals broadcast via a DRAM bounce, applied in place.
            oT = perb1.tile([128, KC, N], BF16, tag="oT")
            for h in range(H):
                po = (h % 2) * 64
                rq, rk = h // 2, H // 2 + h // 2
                qT = qkT[po:po + 64, rq, :]
                pts = []
                for jp in range(NJT // 2):
                    ps2 = s_ps.tile([128, 2, N], F32, tag="sT")
                    for jl in range(2):
                        jt = jp * 2 + jl
                        nc.tensor.matmul(
                            ps2[:, jl, :],
                            qkT[po:po + 64, rk, jt * 128:(jt + 1) * 128],
                            qT,
                            start=True,
                            stop=False,
                        )
                    for jl in range(2):
                        jt = jp * 2 + jl
                        nc.tensor.matmul(
                            ps2[:, jl, :], ident, biasT[:, h, jt, :],
                            start=False, stop=True,
                        )
                    pt2 = pool_p.tile([128, 2, N], BF16, tag="pT")
                    nc.scalar.activation(out=pt2, in_=ps2, func=Exp)
                    pts.append(pt2)
                pso = o_ps.tile([HD + 1, N], F32, tag="oT")
                for jt in range(NJT):
                    vp = v_pad[:, jt, :].rearrange("p (h e) -> p h e", e=HD + 1)
                    nc.tensor.matmul(
                        pso,
                        vp[:, h, :],
                        pts[jt // 2][:, jt % 2, :],
                        start=(jt == 0),
                        stop=(jt == NJT - 1),
                    )
                nc.vector.tensor_copy(out=oT[po:po + 64, rq, :], in_=pso[0:HD, :])
                smc = pool_r.tile([65, N], F32, tag="smc")
                nc.vector.tensor_copy(out=smc[HD:HD + 1, :], in_=pso[HD:HD + 1, :])
                rcd = dram_p.tile([1, N], F32, tag="rcd")
                nc.sync.dma_start(out=rcd, in_=smc[HD:HD + 1, :])
                rcf = pool_r.tile([128, N], F32, tag="rcf")
                nc.sync.dma_start(out=rcf, in_=rcd[0:1, :].to_broadcast((128, N)))
                nc.vector.reciprocal_approx_fast(out=rcf, in_=rcf)
                nc.vector.tensor_mul(
                    oT[po:po + 64, rq, :], oT[po:po + 64, rq, :],
                    rcf[po:po + 64, :],
                )
            return oT

        def proj(b, oT):
            for it in range(NIT):
                outsb = pool_o.tile([128, C], F32, tag="outsb")
                for ct in range(2):
                    ps = mm_ps.tile([128, C // 2], F32, tag="mm")
                    for kc in range(KC):
                        nc.tensor.matmul(
                            ps,
                            oT[:, kc, it * 128:(it + 1) * 128],
                            wp[:, kc, ct * 384:(ct + 1) * 384],
                            start=(kc == 0),
                            stop=(kc == KC - 1),
                        )
                    nc.vector.tensor_add(
                        outsb[:, ct * 384:(ct + 1) * 384],
                        ps,
                        pb_bc[:, ct * 384:(ct + 1) * 384],
                    )
                nc.sync.dma_start(
                    out=out_d[b * N + it * 128: b * N + (it + 1) * 128, :],
                    in_=outsb,
                )

        # software pipeline: keep the PE dense across batch boundaries by
        # running the next batch's projections/qkv before this batch's
        # output projection (which waits on the softmax-normalization tail).
        st = phase1(0)
        for b in range(BL):
            oT = attention(b, *st)
            if b + 1 < BL:
                st = phase1(b + 1)
            proj(b, oT)
    nc.finalize()
    return nc


def kernel(x, qkv_w, proj_w, proj_b, bias_table, _trace=False, _tmpdir=None):
    x = np.asarray(x, dtype=np.float32)
    qkv_w = np.asarray(qkv_w, dtype=np.float32)
    proj_w = np.asarray(proj_w, dtype=np.float32)
    proj_b = np.asarray(proj_b, dtype=np.float32)
    bias_table = np.asarray(bias_table, dtype=np.float32)

    # host-side layout prep (weights + bias table expansion)
    wq_scaled = qkv_w.copy()
    wq_scaled[:C] *= HD ** (-0.5)
    wqk = np.ascontiguousarray(wq_scaled[: 2 * C].T).astype(ml_dtypes.bfloat16)
    wv = np.ascontiguousarray(qkv_w[2 * C:].T).astype(ml_dtypes.bfloat16)
    wp = np.ascontiguousarray(proj_w.T).astype(ml_dtypes.bfloat16)
    ii = np.arange(N)
    idx = ii[None, :] - ii[:, None] + (N - 1)                     # [j, i]
    biasT = np.ascontiguousarray(
        bias_table[idx].transpose(2, 0, 1).reshape(H, NJT, 128, N)
    ).astype(ml_dtypes.bfloat16)
    pb = proj_b.reshape(1, C)

    nc = _build()
    in_maps = []
    for m in range(NCORES):
        xs = x[m * BL:(m + 1) * BL]                               # [8, 512, 768]
        xT = np.ascontiguousarray(xs.transpose(2, 0, 1).reshape(C, T)).astype(ml_dtypes.bfloat16)
        in_maps.append(
            {"xT": xT, "wqk": wqk, "wv": wv, "wp": wp, "biasT": biasT, "pb": pb}
        )
    res = run_bass_kernel_spmd(
        nc, in_maps, core_ids=list(range(NCORES)), trace=_trace, tmpdir=_tmpdir
    )
    out = np.concatenate(
        [res.results[m]["out"].reshape(BL, N, C) for m in range(NCORES)], axis=0
    )
    if _trace:
        return out, res
    return out


# revision 23
# speedup vs baseline: 1.7938x; 1.0865x over previous
"""Swin-style attention (B=64,N=512,C=768,H=12) on 8 TRN2 NeuronCores.

Strategy: pure data-parallel over batch (8 batches/core), no collectives.
Per core, one fused pipeline per batch:
  phase1: qkT = Wqk @ x^T (f32r matmuls), v = x @ Wv^T (natural layout,
          padded with a ones-column per head for fused softmax sums)
  attn:   per head h: sT[j,i] = kT^T@qT (K=64, f32r) -> +biasT (DVE)
          -> exp (ACT, bf16 out) -> oT~[d,i] & sums via [v|1] matmul (bf16)
          -> normalize rows by 1/sums (partition_broadcast + DVE mul)
  proj:   out = oT^T @ Wp^T + pb (bf16 matmul, f32r-grade accuracy not
          needed after softmax averaging)
Scale 1/8 is folded into the q-half of Wqk on the host; softmax runs
without max-subtraction (scores are O(1) by construction).
"""
import sys

sys.path.insert(0, "/opt/trn_rl_repo")
from contextlib import ExitStack

import ml_dtypes
import numpy as np

import concourse.bass as bass
import concourse.mybir as mybir
import concourse.tile as tile
from concourse import bacc
from concourse.bass_utils import run_bass_kernel_spmd
from concourse.masks import make_identity

F32 = mybir.dt.float32
F32R = mybir.dt.float32r
BF16 = mybir.dt.bfloat16

B, N, C, H, HD = 64, 512, 768, 12, 64
NCORES = 8
BL = B // NCORES          # batches per core
T = BL * N                # tokens per core
KC = C // 128             # 6 contraction chunks
NJT = N // 128            # 4 key-side tiles
NIT = N // 128            # 4 query/token tiles
VP = H * (HD + 1)         # 780: v padded with ones column per head
Exp = mybir.ActivationFunctionType.Exp


def _build():
    nc = bacc.Bacc(target_bir_lowering=False)
    xT_d = nc.dram_tensor("xT", [C, T], BF16, kind="ExternalInput")
    wqk_d = nc.dram_tensor("wqk", [C, 2 * C], BF16, kind="ExternalInput")
    wv_d = nc.dram_tensor("wv", [C, C], BF16, kind="ExternalInput")
    wp_d = nc.dram_tensor("wp", [C, C], BF16, kind="ExternalInput")
    biasT_d = nc.dram_tensor("biasT", [H, NJT, 128, N], BF16, kind="ExternalInput")
    pb_d = nc.dram_tensor("pb", [1, C], F32, kind="ExternalInput")
    out_d = nc.dram_tensor("out", [T, C], F32, kind="ExternalOutput")

    with ExitStack() as ctx:
        tc = ctx.enter_context(tile.TileContext(nc))
        const = ctx.enter_context(tc.tile_pool(name="const", bufs=1))
        perb = ctx.enter_context(tc.tile_pool(name="perb", bufs=2))
        perb1 = ctx.enter_context(tc.tile_pool(name="perb1", bufs=2))
        xt_pool = ctx.enter_context(tc.tile_pool(name="xt", bufs=2))
        pool_p = ctx.enter_context(tc.tile_pool(name="pt", bufs=5))
        pool_r = ctx.enter_context(tc.tile_pool(name="rc", bufs=3))
        pool_o = ctx.enter_context(tc.tile_pool(name="osb", bufs=2))
        dram_p = ctx.enter_context(tc.tile_pool(name="dramp", bufs=2, space="DRAM"))
        mm_ps = ctx.enter_context(tc.tile_pool(name="mmps", bufs=2, space="PSUM"))
        s_ps = ctx.enter_context(tc.tile_pool(name="sps", bufs=2, space="PSUM"))
        o_ps = ctx.enter_context(tc.tile_pool(name="ops", bufs=2, space="PSUM"))

        # ---- constants ----
        wqk = const.tile([128, KC, 2 * C], BF16)
        wv = const.tile([128, KC, C], BF16)
        wp = const.tile([128, KC, C], BF16)
        biasT = const.tile([128, H, NJT, N], BF16)
        pb_bc = const.tile([128, C], F32)
        ident = const.tile([128, 128], BF16)
        make_identity(nc, ident)
        for kc in range(KC):
            nc.sync.dma_start(out=wqk[:, kc, :], in_=wqk_d[kc * 128:(kc + 1) * 128, :])
            nc.sync.dma_start(out=wv[:, kc, :], in_=wv_d[kc * 128:(kc + 1) * 128, :])
            nc.sync.dma_start(out=wp[:, kc, :], in_=wp_d[kc * 128:(kc + 1) * 128, :])
        for h in range(H):
            nc.sync.dma_start(
                out=biasT[:, h, :, :],
                in_=biasT_d[h, :, :, :].rearrange("a p b -> p a b"),
            )
        nc.sync.dma_start(out=pb_bc, in_=pb_d[0:1, :].to_broadcast((128, C)))

        def phase1(b):
            # ---- load x^T for this batch ----
            xTb = xt_pool.tile([128, KC, N], BF16, tag="xTb")
            for kc in range(KC):
                nc.sync.dma_start(
                    out=xTb[:, kc, :],
                    in_=xT_d[kc * 128:(kc + 1) * 128, b * N:(b + 1) * N],
                )

            # ---- phase 1a: qkT[r, i] for r in 2C (q rows then k rows) ----
            qkT = perb.tile([128, 2 * H // 2, N], BF16, tag="qkT")
            for rt in range(2 * H // 2):  # 12 r-tiles of 128
                ps = mm_ps.tile([128, N], F32, tag="mm")
                for kc in range(KC):
                    nc.tensor.matmul(
                        ps,
                        wqk[:, kc, rt * 128:(rt + 1) * 128],
                        xTb[:, kc, :],
                        start=(kc == 0),
                        stop=(kc == KC - 1),
                    )
                nc.scalar.copy(out=qkT[:, rt, :], in_=ps)

            # ---- phase 1b: v natural, padded with ones column per head ----
            v_pad = perb.tile([128, NIT, VP], BF16, tag="v_pad")
            ones_view = v_pad.rearrange("p a (h e) -> p (a h) e", e=HD + 1)
            nc.vector.memset(ones_view[:, :, HD:HD + 1], 1.0)
            for it in range(NIT):
                for nh in range(2):
                    ps = mm_ps.tile([128, C // 2], F32, tag="mm")
                    for kc in range(KC):
                        nc.tensor.matmul(
                            ps,
                            xTb[:, kc, it * 128:(it + 1) * 128],
                            wv[:, kc, nh * 384:(nh + 1) * 384],
                            start=(kc == 0),
                            stop=(kc == KC - 1),
                        )
                    dest = v_pad[:, it, :].rearrange("p (h e) -> p h e", e=HD + 1)
                    nc.scalar.copy(
                        out=dest[:, nh * 6:(nh + 1) * 6, 0:HD],
                        in_=ps.rearrange("p (h e) -> p h e", e=HD),
                    )
            return qkT, v_pad

        def attention(b, qkT, v_pad):
            # scores: sT = kT^T@qT accumulated with I^T@biasT (bias add on PE);
            # exp straight from PSUM; softmax sums ride as row 64 of the
            # o-matmul via the ones column of v_pad.  Normalization is
            # deferred: unnormalized oT rows are stored bf16, per-head
            # reciprocals broadcast via a DRAM bounce, applied in place.
            oT = perb1.tile([128, KC, N], BF16, tag="oT")
            for h in range(H):
                po = (h % 2) * 64
                rq, rk = h // 2, H // 2 + h // 2
                qT = qkT[po:po + 64, rq, :]
                pts = []
                for jp in range(NJT // 2):
                    ps2 = s_ps.tile([128, 2, N], F32, tag="sT")
                    for jl in range(2):
                        jt = jp * 2 + jl
                        nc.tensor.matmul(
                            ps2[:, jl, :],
                            qkT[po:po + 64, rk, jt * 128:(jt + 1) * 128],
                            qT,
                            start=True,
                            stop=False,
                        )
                    for jl in range(2):
                        jt = jp * 2 + jl
                        nc.tensor.matmul(
                            ps2[:, jl, :], ident, biasT[:, h, jt, :],
                            start=False, stop=True,
                        )
                    pt2 = pool_p.tile([128, 2, N], BF16, tag="pT")
                    nc.scalar.activation(out=pt2, in_=ps2, func=Exp)
                    pts.append(pt2)
                pso = o_ps.tile([HD + 1, N], F32, tag="oT")
                for jt in range(NJT):
                    vp = v_pad[:, jt, :].rearrange("p (h e) -> p h e", e=HD + 1)
                    nc.tensor.matmul(
                        pso,
                        vp[:, h, :],
                        pts[jt // 2][:, jt % 2, :],
                        start=(jt == 0),
                        stop=(jt == NJT - 1),
                    )
                nc.vector.tensor_copy(out=oT[po:po + 64, rq, :], in_=pso[0:HD, :])
                smc = pool_r.tile([65, N], F32, tag="smc")
                nc.vector.tensor_copy(out=smc[HD:HD + 1, :], in_=pso[HD:HD + 1, :])
                rcd = dram_p.tile([1, N], F32, tag="rcd")
                nc.sync.dma_start(out=rcd, in_=smc[HD:HD + 1, :])
                rcf = pool_r.tile([128, N], F32, tag="rcf")
                nc.sync.dma_start(out=rcf, in_=rcd[0:1, :].to_broadcast((128, N)))
                nc.vector.reciprocal_approx_fast(out=rcf, in_=rcf)
                nc.vector.tensor_mul(
                    oT[po:po + 64, rq, :], oT[po:po + 64, rq, :],
                    rcf[po:po + 64, :],
                )
            return oT

        def proj(b, oT):
            for it in range(NIT):
                outsb = pool_o.tile([128, C], F32, tag="outsb")
                for ct in range(2):
                    ps = mm_ps.tile([128, C // 2], F32, tag="mm")
                    for kc in range(KC):
                        nc.tensor.matmul(
                            ps,
                            oT[:, kc, it * 128:(it + 1) * 128],
                            wp[:, kc, ct * 384:(ct + 1) * 384],
                            start=(kc == 0),
                            stop=(kc == KC - 1),
                        )
                    nc.vector.tensor_add(
                        outsb[:, ct * 384:(ct + 1) * 384],
                        ps,
                        pb_bc[:, ct * 384:(ct + 1) * 384],
                    )
                nc.sync.dma_start(
                    out=out_d[b * N + it * 128: b * N + (it + 1) * 128, :],
                    in_=outsb,
                )

        # software pipeline: keep the PE dense across batch boundaries by
        # running the next batch's projections/qkv before this batch's
        # output projection (which waits on the softmax-normalization tail).
        st = phase1(0)
        for b in range(BL):
            oT = attention(b, *st)
            if b + 1 < BL:
                st = phase1(b + 1)
            proj(b, oT)
    nc.finalize()
    return nc


def kernel(x, qkv_w, proj_w, proj_b, bias_table, _trace=False, _tmpdir=None):
    x = np.asarray(x, dtype=np.float32)
    qkv_w = np.asarray(qkv_w, dtype=np.float32)
    proj_w = np.asarray(proj_w, dtype=np.float32)
    proj_b = np.asarray(proj_b, dtype=np.float32)
    bias_table = np.asarray(bias_table, dtype=np.float32)

    # host-side layout prep (weights + bias table expansion)
    wq_scaled = qkv_w.copy()
    wq_scaled[:C] *= HD ** (-0.5)
    wqk = np.ascontiguousarray(wq_scaled[: 2 * C].T).astype(ml_dtypes.bfloat16)
    wv = np.ascontiguousarray(qkv_w[2 * C:].T).astype(ml_dtypes.bfloat16)
    wp = np.ascontiguousarray(proj_w.T).astype(ml_dtypes.bfloat16)
    ii = np.arange(N)
    idx = ii[None, :] - ii[:, None] + (N - 1)                     # [j, i]
    biasT = np.ascontiguousarray(
        bias_table[idx].transpose(2, 0, 1).reshape(H, NJT, 128, N)
    ).astype(ml_dtypes.bfloat16)
    pb = proj_b.reshape(1, C)

    nc = _build()
    in_maps = []
    for m in range(NCORES):
        xs = x[m * BL:(m + 1) * BL]                               # [8, 512, 768]
        xT = np.ascontiguousarray(xs.transpose(2, 0, 1).reshape(C, T)).astype(ml_dtypes.bfloat16)
        in_maps.append(
            {"xT": xT, "wqk": wqk, "wv": wv, "wp": wp, "biasT": biasT, "pb": pb}
        )
    res = run_bass_kernel_spmd(
        nc, in_maps, core_ids=list(range(NCORES)), trace=_trace, tmpdir=_tmpdir
    )
    out = np.concatenate(
        [res.results[m]["out"].reshape(BL, N, C) for m in range(NCORES)], axis=0
    )
    if _trace:
        return out, res
    return out


# revision 28
# speedup vs baseline: 1.8240x; 1.0168x over previous
"""Swin-style attention (B=64,N=512,C=768,H=12) on 8 TRN2 NeuronCores.

Strategy: pure data-parallel over batch (8 batches/core), no collectives.
Per core, one fused pipeline per batch:
  phase1: qkT = Wqk @ x^T (f32r matmuls), v = x @ Wv^T (natural layout,
          padded with a ones-column per head for fused softmax sums)
  attn:   per head h: sT[j,i] = kT^T@qT (K=64, f32r) -> +biasT (DVE)
          -> exp (ACT, bf16 out) -> oT~[d,i] & sums via [v|1] matmul (bf16)
          -> normalize rows by 1/sums (partition_broadcast + DVE mul)
  proj:   out = oT^T @ Wp^T + pb (bf16 matmul, f32r-grade accuracy not
          needed after softmax averaging)
Scale 1/8 is folded into the q-half of Wqk on the host; softmax runs
without max-subtraction (scores are O(1) by construction).
"""
import sys

sys.path.insert(0, "/opt/trn_rl_repo")
from contextlib import ExitStack

import ml_dtypes
import numpy as np

import concourse.bass as bass
import concourse.mybir as mybir
import concourse.tile as tile
from concourse import bacc
from concourse.bass_utils import run_bass_kernel_spmd
from concourse.masks import make_identity

F32 = mybir.dt.float32
F32R = mybir.dt.float32r
BF16 = mybir.dt.bfloat16

B, N, C, H, HD = 64, 512, 768, 12, 64
NCORES = 8
BL = B // NCORES          # batches per core
T = BL * N                # tokens per core
KC = C // 128             # 6 contraction chunks
NJT = N // 128            # 4 key-side tiles
NIT = N // 128            # 4 query/token tiles
VP = H * (HD + 1)         # 780: v padded with ones column per head
Exp = mybir.ActivationFunctionType.Exp


def _build():
    nc = bacc.Bacc(target_bir_lowering=False)
    xT_d = nc.dram_tensor("xT", [C, T], BF16, kind="ExternalInput")
    wqk_d = nc.dram_tensor("wqk", [C, 2 * C], BF16, kind="ExternalInput")
    wv_d = nc.dram_tensor("wv", [C, C], BF16, kind="ExternalInput")
    wp_d = nc.dram_tensor("wp", [C, C], BF16, kind="ExternalInput")
    biasT_d = nc.dram_tensor("biasT", [H, NJT, 128, N], BF16, kind="ExternalInput")
    pb_d = nc.dram_tensor("pb", [1, C], F32, kind="ExternalInput")
    out_d = nc.dram_tensor("out", [T, C], F32, kind="ExternalOutput")

    with ExitStack() as ctx:
        tc = ctx.enter_context(tile.TileContext(nc))
        const = ctx.enter_context(tc.tile_pool(name="const", bufs=1))
        perb = ctx.enter_context(tc.tile_pool(name="perb", bufs=2))
        perb1 = ctx.enter_context(tc.tile_pool(name="perb1", bufs=2))
        xt_pool = ctx.enter_context(tc.tile_pool(name="xt", bufs=2))
        pool_p = ctx.enter_context(tc.tile_pool(name="pt", bufs=3))
        pool_r = ctx.enter_context(tc.tile_pool(name="rc", bufs=3))
        pool_o = ctx.enter_context(tc.tile_pool(name="osb", bufs=2))
        dram_p = ctx.enter_context(tc.tile_pool(name="dramp", bufs=2, space="DRAM"))
        mm_ps = ctx.enter_context(tc.tile_pool(name="mmps", bufs=2, space="PSUM"))
        s_ps = ctx.enter_context(tc.tile_pool(name="sps", bufs=1, space="PSUM"))
        o_ps = ctx.enter_context(tc.tile_pool(name="ops", bufs=2, space="PSUM"))

        # ---- constants ----
        wqk = const.tile([128, KC, 2 * C], BF16)
        wv = const.tile([128, KC, C], BF16)
        wp = const.tile([128, KC, C], BF16)
        biasT = const.tile([128, H, NJT, N], BF16)
        pb_bc = const.tile([128, C], F32)
        ident = const.tile([128, 128], BF16)
        make_identity(nc, ident)
        def load_consts_early():
            for kc in range(KC):
                nc.sync.dma_start(
                    out=wqk[:, kc, :], in_=wqk_d[kc * 128:(kc + 1) * 128, :]
                )
                nc.sync.dma_start(
                    out=wv[:, kc, :], in_=wv_d[kc * 128:(kc + 1) * 128, :]
                )

        def load_consts_late():
            for kc in range(KC):
                nc.sync.dma_start(
                    out=wp[:, kc, :], in_=wp_d[kc * 128:(kc + 1) * 128, :]
                )
            for h in range(H):
                nc.sync.dma_start(
                    out=biasT[:, h, :, :],
                    in_=biasT_d[h, :, :, :].rearrange("a p b -> p a b"),
                )
            nc.sync.dma_start(out=pb_bc, in_=pb_d[0:1, :].to_broadcast((128, C)))

        def load_x(b):
            xTb = xt_pool.tile([128, KC, N], BF16, tag="xTb")
            for kc in range(KC):
                nc.sync.dma_start(
                    out=xTb[:, kc, :],
                    in_=xT_d[kc * 128:(kc + 1) * 128, b * N:(b + 1) * N],
                )
            return xTb

        def alloc_qkT():
            return perb.tile([128, 2 * H // 2, N], BF16, tag="qkT", name="qkT")

        def alloc_vpad():
            v_pad = perb.tile([128, NIT, VP], BF16, tag="v_pad")
            ones_view = v_pad.rearrange("p a (h e) -> p (a h) e", e=HD + 1)
            nc.vector.memset(ones_view[:, :, HD:HD + 1], 1.0)
            return v_pad

        def qk_tile(qkT, xTb, rt):
            ps = mm_ps.tile([128, N], F32, tag="mm")
            for kc in range(KC):
                nc.tensor.matmul(
                    ps,
                    wqk[:, kc, rt * 128:(rt + 1) * 128],
                    xTb[:, kc, :],
                    start=(kc == 0),
                    stop=(kc == KC - 1),
                )
            nc.vector.tensor_copy(out=qkT[:, rt, :], in_=ps)

        def v_tile(v_pad, xTb, it, nh):
            ps = mm_ps.tile([128, C // 2], F32, tag="mm")
            for kc in range(KC):
                nc.tensor.matmul(
                    ps,
                    xTb[:, kc, it * 128:(it + 1) * 128],
                    wv[:, kc, nh * 384:(nh + 1) * 384],
                    start=(kc == 0),
                    stop=(kc == KC - 1),
                )
            dest = v_pad[:, it, :].rearrange("p (h e) -> p h e", e=HD + 1)
            nc.vector.tensor_copy(
                out=dest[:, nh * 6:(nh + 1) * 6, 0:HD],
                in_=ps.rearrange("p (h e) -> p h e", e=HD),
            )

        def proj_tile(b, oT, it):
            outsb = pool_o.tile([128, C], F32, tag="outsb")
            for ct in range(2):
                ps = mm_ps.tile([128, C // 2], F32, tag="mm")
                for kc in range(KC):
                    nc.tensor.matmul(
                        ps,
                        oT[:, kc, it * 128:(it + 1) * 128],
                        wp[:, kc, ct * 384:(ct + 1) * 384],
                        start=(kc == 0),
                        stop=(kc == KC - 1),
                    )
                nc.vector.tensor_add(
                    outsb[:, ct * 384:(ct + 1) * 384],
                    ps,
                    pb_bc[:, ct * 384:(ct + 1) * 384],
                )
            nc.sync.dma_start(
                out=out_d[b * N + it * 128: b * N + (it + 1) * 128, :],
                in_=outsb,
            )

        def pair_scores(qkT, hp, jp):
            # head pair (hA even -> PE rows 0-63, hB odd -> rows 64-127):
            # their K=64 score matmuls execute concurrently on disjoint
            # row groups of the systolic array; I^T@biasT rides the same
            # PSUM accumulation; exp straight from PSUM, 1024 wide.
            rq, rk = hp, H // 2 + hp
            psA = s_ps.tile([128, 2, N], F32, tag="sTA")
            psB = s_ps.tile([128, 2, N], F32, tag="sTB")
            for jl in range(2):
                jt = jp * 2 + jl
                nc.tensor.matmul(
                    psA[:, jl, :],
                    qkT[0:64, rk, jt * 128:(jt + 1) * 128],
                    qkT[0:64, rq, :],
                    start=True,
                    stop=False,
                )
                nc.tensor.matmul(
                    psB[:, jl, :],
                    qkT[64:128, rk, jt * 128:(jt + 1) * 128],
                    qkT[64:128, rq, :],
                    start=True,
                    stop=False,
                )
            for jl in range(2):
                jt = jp * 2 + jl
                nc.tensor.matmul(
                    psA[:, jl, :], ident, biasT[:, 2 * hp, jt, :],
                    start=False, stop=True,
                )
                nc.tensor.matmul(
                    psB[:, jl, :], ident, biasT[:, 2 * hp + 1, jt, :],
                    start=False, stop=True,
                )
            ptA = pool_p.tile([128, 2, N], BF16, tag="pTA")
            nc.scalar.activation(out=ptA, in_=psA, func=Exp)
            ptB = pool_p.tile([128, 2, N], BF16, tag="pTB")
            nc.scalar.activation(out=ptB, in_=psB, func=Exp)
            return ptA, ptB

        def head_out(oT, v_pad, h, pts):
            po = (h % 2) * 64
            rqo = h // 2
            pso = o_ps.tile([HD + 1, N], F32, tag="oT")
            for jt in range(NJT):
                vp = v_pad[:, jt, :].rearrange("p (h e) -> p h e", e=HD + 1)
                nc.tensor.matmul(
                    pso,
                    vp[:, h, :],
                    pts[jt // 2][:, jt % 2, :],
                    start=(jt == 0),
                    stop=(jt == NJT - 1),
                )
            nc.vector.tensor_copy(out=oT[po:po + 64, rqo, :], in_=pso[0:HD, :])
            smc = pool_r.tile([65, N], F32, tag="smc")
            nc.vector.tensor_copy(out=smc[HD:HD + 1, :], in_=pso[HD:HD + 1, :])
            rcd = dram_p.tile([1, N], F32, tag="rcd")
            nc.sync.dma_start(out=rcd, in_=smc[HD:HD + 1, :])
            rcf = pool_r.tile([128, N], F32, tag="rcf")
            nc.sync.dma_start(out=rcf, in_=rcd[0:1, :].to_broadcast((128, N)))
            nc.vector.reciprocal_approx_fast(out=rcf, in_=rcf)
            nc.vector.tensor_mul(
                oT[po:po + 64, rqo, :], oT[po:po + 64, rqo, :],
                rcf[po:po + 64, :],
            )

        # ---- software-pipelined schedule ----
        # Per batch: 6 head pairs; between a pair's score matmuls and its
        # o-matmuls (which wait on ACT exps), the PE is fed filler work from
        # the NEXT batch's qkv projection; v/proj tiles trail each batch.
        load_consts_early()
        xT_cur = load_x(0)
        qkT_cur = alloc_qkT()
        vp_cur = alloc_vpad()
        for rt in range(12):
            qk_tile(qkT_cur, xT_cur, rt)
        load_consts_late()
        for it in range(NIT):
            for nh in range(2):
                v_tile(vp_cur, xT_cur, it, nh)

        oT_prev, b_prev = None, None
        for b in range(BL):
            qkT, v_pad = qkT_cur, vp_cur
            fillers = []
            if b + 1 < BL:
                xT_nxt = load_x(b + 1)
                qkT_cur = alloc_qkT()
                vp_cur = alloc_vpad()
                fillers += [
                    (lambda rt=rt: qk_tile(qkT_cur, xT_nxt, rt))
                    for rt in range(12)
                ]
                fillers += [
                    (lambda it=it, nh=nh: v_tile(vp_cur, xT_nxt, it, nh))
                    for it in range(NIT) for nh in range(2)
                ]
            if oT_prev is not None:
                fillers += [
                    (lambda it=it: proj_tile(b_prev, oT_prev, it))
                    for it in range(NIT)
                ]
            fi = 0
            oT = perb1.tile([128, KC, N], BF16, tag="oT")
            for hp in range(H // 2):
                pts01 = pair_scores(qkT, hp, 0)
                if fi < len(fillers):
                    fillers[fi](); fi += 1
                pts23 = pair_scores(qkT, hp, 1)
                if fi < len(fillers):
                    fillers[fi](); fi += 1
                ptsA = [pts01[0], pts23[0]]
                ptsB = [pts01[1], pts23[1]]
                head_out(oT, v_pad, 2 * hp, ptsA)
                head_out(oT, v_pad, 2 * hp + 1, ptsB)
            for f in fillers[fi:]:
                f()
            oT_prev, b_prev = oT, b
        for it in range(NIT):
            proj_tile(b_prev, oT_prev, it)
    nc.finalize()
    return nc


def kernel(x, qkv_w, proj_w, proj_b, bias_table, _trace=False, _tmpdir=None):
    x = np.asarray(x, dtype=np.float32)
    qkv_w = np.asarray(qkv_w, dtype=np.float32)
    proj_w = np.asarray(proj_w, dtype=np.float32)
    proj_b = np.asarray(proj_b, dtype=np.float32)
    bias_table = np.asarray(bias_table, dtype=np.float32)

    # host-side layout prep (weights + bias table expansion)
    wq_scaled = qkv_w.copy()
    wq_scaled[:C] *= HD ** (-0.5)
    wqk = np.ascontiguousarray(wq_scaled[: 2 * C].T).astype(ml_dtypes.bfloat16)
    wv = np.ascontiguousarray(qkv_w[2 * C:].T).astype(ml_dtypes.bfloat16)
    wp = np.ascontiguousarray(proj_w.T).astype(ml_dtypes.bfloat16)
    ii = np.arange(N)
    idx = ii[None, :] - ii[:, None] + (N - 1)                     # [j, i]
    biasT = np.ascontiguousarray(
        bias_table[idx].transpose(2, 0, 1).reshape(H, NJT, 128, N)
    ).astype(ml_dtypes.bfloat16)
    pb = proj_b.reshape(1, C)

    nc = _build()
    in_maps = []
    for m in range(NCORES):
        xs = x[m * BL:(m + 1) * BL]                               # [8, 512, 768]
        xT = np.ascontiguousarray(xs.transpose(2, 0, 1).reshape(C, T)).astype(ml_dtypes.bfloat16)
        in_maps.append(
            {"xT": xT, "wqk": wqk, "wv": wv, "wp": wp, "biasT": biasT, "pb": pb}
        )
    res = run_bass_kernel_spmd(
        nc, in_maps, core_ids=list(range(NCORES)), trace=_trace, tmpdir=_tmpdir
    )
    out = np.concatenate(
        [res.results[m]["out"].reshape(BL, N, C) for m in range(NCORES)], axis=0
    )
    if _trace:
        return out, res
    return out
